# revision 17
# baseline (speedup 1.0000x reference)
"""Trainium2 Bass kernel for nn_CrossAttention_79362405696071.

Reference computation (per batch b):
  iq/ik/iv = img @ W_i{q,k,v}; cq/ck/cv = clinic @ W_c{q,k,v}   (8 heads x 64)
  k/v get one zero-padded key slot appended.
  img_out  = ((attend(iq,ck,cv) + attend(iq,ik,iv)) * 0.5) @ W_io + b_io
  clin_out = ((attend(cq,ik,iv) + attend(cq,ck,cv)) * 0.5) @ W_co + b_co

Sharding: 8 cores = batch(2) x head-pairs(4). Core c handles batch c//4 and
heads {2*(c%4), 2*(c%4)+1} (a 128-wide slice of the inner dim). Each core
computes both modalities' attention for its heads and the partial output
projection (contraction over its 128 inner dims). The 4 partials per batch
are summed on the host (host-side all-reduce), and host adds the bias.

Per-core structure (identical SPMD program, different data):
  xT[mod]    [512(dim), 2048(n)]  - host-transposed activations
  qT/kT      [128(pair-inner), 2048] = W_slice.T @ xT            (PE)
  v_ext      [2048(keys), per kt: v_h0(64)|1|v_h1(64)|1]  (v pre-scaled 0.5,
             with a ones column appended per head for the softmax denominator)
  dotsT      [128(key-tile), 2x512(h,q)] PSUM = k_h.T @ q_h (row-packed pair)
  expT       = exp(0.125 * dotsT)        (ACT, free-dim 1024)
  av_ext[bi][h] [65, 512] PSUM += [v_h|1].T @ expT_h  over 16 key tiles;
             row 64 accumulates the softmax denominator.
  combined_h = av_self_h * 1/(den_s+1) + av_cross_h * 1/(den_c+1)   (DVE)
             (+1 = zero-padded key slot: exp(0) in the denominator)
  out_part  += combined_h(128-col slices).T @ Wo_h  -> DRAM f32 partial
"""

import os
import sys

for _p in ("/opt/trn_rl_repo",):
    if _p not in sys.path:
        sys.path.insert(0, _p)

import numpy as np
import ml_dtypes
from contextlib import ExitStack

import concourse.bass as bass
import concourse.bacc as bacc
import concourse.tile as tile
import concourse.mybir as mybir

BF16 = mybir.dt.bfloat16
F32 = mybir.dt.float32
F32R = mybir.dt.float32r
EXP = mybir.ActivationFunctionType.Exp
NPBF16 = ml_dtypes.bfloat16

P = 128          # partitions
N = 2048         # sequence length (both modalities)
DIM = 512        # model dim
DC = DIM // P    # 4 dim chunks
KT = N // P      # 16 key tiles
QB = N // 512    # 4 query blocks of 512
VW = 130         # per-kt width in v_ext: 64 + 1 + 64 + 1
SCALE = 64 ** -0.5  # 1/8

# compute dtype mode: "bf16" or "f32r"
DTYPE_MODE = os.environ.get("KERNEL_DTYPE", "bf16")
FILLERS = int(os.environ.get("KERNEL_FILLERS", "0"))


def _build(dtype_mode):
    """Build + compile the single-core SPMD program. Returns nc."""
    cdt = BF16 if dtype_mode == "bf16" else F32
    mm = (lambda ap: ap) if dtype_mode == "bf16" else (lambda ap: ap.bitcast(F32R))

    nc = bacc.Bacc("TRN2", target_bir_lowering=False, debug=False)

    mods = ("a", "b")
    xt_d = {m: nc.declare_dram_parameter(f"xt_{m}", [DIM, N], cdt, isOutput=False).ap() for m in mods}
    wq_d = {m: nc.declare_dram_parameter(f"wq_{m}", [DIM, P], cdt, isOutput=False).ap() for m in mods}
    wk_d = {m: nc.declare_dram_parameter(f"wk_{m}", [DIM, P], cdt, isOutput=False).ap() for m in mods}
    wv_d = {m: nc.declare_dram_parameter(f"wv_{m}", [DIM, P], cdt, isOutput=False).ap() for m in mods}
    wo_d = {m: nc.declare_dram_parameter(f"wo_{m}", [P, DIM], cdt, isOutput=False).ap() for m in mods}
    out_d = {m: nc.declare_dram_parameter(f"out_{m}", [N, DIM], F32, isOutput=True).ap() for m in mods}

    with tile.TileContext(nc) as tc, ExitStack() as ctx:
        persist = ctx.enter_context(tc.tile_pool(name="persist", bufs=1))
        exp_pool = ctx.enter_context(tc.tile_pool(name="exp", bufs=12))
        misc = ctx.enter_context(tc.tile_pool(name="misc", bufs=2))
        ps_big = ctx.enter_context(tc.tile_pool(name="ps_big", bufs=2, space="PSUM"))
        ps_av = ctx.enter_context(tc.tile_pool(name="ps_av", bufs=4, space="PSUM"))

        # ---- load inputs to SBUF ----
        xt = {}   # [mod][chunk] -> [128, 2048]
        wq, wk, wv, wo = {}, {}, {}, {}
        for m in mods:
            xt[m] = []
            xr = xt_d[m].rearrange("(c p) n -> c p n", p=P)
            for c in range(DC):
                t = persist.tile([P, N], cdt, name=f"xt_{m}_{c}", tag=f"xt_{m}_{c}")
                nc.sync.dma_start(t[:], xr[c])
                xt[m].append(t)
            for wname, w, d in (("wq", wq, wq_d), ("wk", wk, wk_d), ("wv", wv, wv_d)):
                w[m] = persist.tile([P, DC * P], cdt, name=f"{wname}_{m}", tag=f"{wname}_{m}")
                nc.sync.dma_start(w[m][:].rearrange("p (c j) -> p c j", j=P),
                                  d[m].rearrange("(c p) j -> p c j", p=P))
            # wo stored as [64, 2, 512]: row j of head-half hh = wo[hh*64+j, :]
            wo[m] = persist.tile([64, 2, DIM], cdt, name=f"wo_{m}", tag=f"wo_{m}")
            nc.sync.dma_start(wo[m][:], wo_d[m].rearrange("(hh p) f -> p hh f", p=64))

        ones64 = persist.tile([1, 64], F32, name="ones64", tag="ones64")
        nc.vector.memset(ones64[:], 1.0)

        # ---- projections ----
        qT, kT, vsb = {}, {}, {}
        for m in mods:
            qT[m] = persist.tile([P, N], cdt, name=f"qT_{m}", tag=f"qT_{m}")
            kT[m] = persist.tile([P, N], cdt, name=f"kT_{m}", tag=f"kT_{m}")
            for w, dst in ((wq, qT), (wk, kT)):
                for nb in range(QB):
                    ps = ps_av.tile([P, 512], F32, tag="av")
                    for c in range(DC):
                        nc.tensor.matmul(
                            ps[:],
                            mm(w[m][:, c * P:(c + 1) * P]),
                            mm(xt[m][c][:, nb * 512:(nb + 1) * 512]),
                            start=(c == 0), stop=(c == DC - 1),
                        )
                    nc.vector.tensor_copy(dst[m][:, nb * 512:(nb + 1) * 512], ps[:])
            # v_ext: [keys, per kt: v_h0|1|v_h1|1], v pre-scaled by 0.5 on host
            vsb[m] = persist.tile([P, KT * VW], cdt, name=f"v_{m}", tag=f"v_{m}")
            nc.vector.memset(vsb[m][:], 1.0)
            for kt in range(KT):
                ps = ps_av.tile([P, P], F32, tag="av")
                for c in range(DC):
                    nc.tensor.matmul(
                        ps[:],
                        mm(xt[m][c][:, kt * P:(kt + 1) * P]),
                        mm(wv[m][:, c * P:(c + 1) * P]),
                        start=(c == 0), stop=(c == DC - 1),
                    )
                for h in range(2):
                    nc.vector.tensor_copy(
                        vsb[m][:, kt * VW + h * 65: kt * VW + h * 65 + 64],
                        ps[:, h * 64:(h + 1) * 64])

        # ---- attention + output projection: flat software pipeline ----
        # One stream over (mod, qb, branch, kt). The av/denominator matmuls
        # trail the dots+exp stream by LAG steps so the ACT engine (the
        # bottleneck) never stalls at branch/combo boundaries; normalize and
        # the output projection of combo j flow during combo j+1.
        LAG = 3
        steps = []
        for m in mods:
            other = "b" if m == "a" else "a"
            for qb in range(QB):
                for bi, kv in enumerate((m, other)):   # 0=self, 1=cross
                    for kt in range(KT):
                        steps.append((m, other, qb, bi, kv, kt))
        exd = {}     # step index -> ex tile
        avd = {}     # (m, qb, bi) -> {h: av psum tile}
        avsbd = {}   # (m, qb, bi) -> {h: drained sbuf tile}
        rdd = {}     # (m, qb, bi) -> {h: rd psum tile}

        def emit_branch_prep(m, qb, bi):
            # drain av psum -> SBUF, build reciprocal row, spread across
            # partitions for a lane-parallel reciprocal, gather back, and
            # K=1 ones-matmul broadcast to [64, 512]
            # (gpsimd.partition_broadcast is broken on HW; DVE cannot cross
            # partitions; +1 accounts for the zero-padded key slot)
            av = avd[(m, qb, bi)]
            avsbd[(m, qb, bi)] = {}
            dnrow = misc.tile([65, 1024], F32, name=f"dnrow{bi}", tag=f"dnrow{bi}")
            for h in range(2):
                nc.vector.tensor_scalar_add(
                    dnrow[64:65, h * 512:(h + 1) * 512], av[h][64:65, :], 1.0)
                t = misc.tile([64, 512], F32, name=f"avsb{bi}{h}", tag=f"avsb{bi}{h}")
                nc.vector.tensor_copy(t[:], av[h][0:64, :])
                avsbd[(m, qb, bi)][h] = t
            dsp = misc.tile([P, 8], F32, name=f"dsp{bi}", tag=f"dsp{bi}")
            nc.gpsimd.dma_start(dsp[:], dnrow[64:65, :])
            rsp = misc.tile([P, 8], F32, name=f"rsp{bi}", tag=f"rsp{bi}")
            nc.vector.reciprocal(rsp[:], dsp[:])
            r0 = misc.tile([1, 1024], F32, name=f"r0{bi}", tag=f"r0{bi}")
            nc.gpsimd.dma_start(r0[:], rsp[:])
            rdd[(m, qb, bi)] = {}
            for h in range(2):
                rd_ps = ps_big.tile([64, 512], F32, tag="dt")
                nc.tensor.matmul(rd_ps[:], ones64[:].bitcast(F32R),
                                 r0[:, h * 512:(h + 1) * 512].bitcast(F32R),
                                 start=True, stop=True)
                rdd[(m, qb, bi)][h] = rd_ps

        def emit_finalize(m, qb):
            avn = misc.tile([64, 2, 512], cdt, name="avn", tag="avn")
            for h in range(2):
                tmp0 = misc.tile([64, 512], F32, name="tmp0", tag="tmp0")
                tmp1 = misc.tile([64, 512], F32, name="tmp1", tag="tmp1")
                nc.vector.tensor_mul(tmp0[:], avsbd[(m, qb, 0)][h][:], rdd[(m, qb, 0)][h][:])
                nc.vector.tensor_mul(tmp1[:], avsbd[(m, qb, 1)][h][:], rdd[(m, qb, 1)][h][:])
                nc.vector.tensor_add(avn[:, h, :], tmp0[:], tmp1[:])
            del avsbd[(m, qb, 0)], avsbd[(m, qb, 1)], rdd[(m, qb, 0)], rdd[(m, qb, 1)]
            osb = misc.tile([P, 4 * 512], F32, name="osb", tag="osb")
            q0 = qb * 512
            for nt in range(4):
                po = ps_av.tile([P, 512], F32, tag="av")
                for h in range(2):
                    nc.tensor.matmul(
                        po[:],
                        mm(avn[:, h, nt * P:(nt + 1) * P]),
                        mm(wo[m][:, h, :]),
                        start=(h == 0), stop=(h == 1),
                    )
                nc.vector.tensor_copy(osb[:, nt * 512:(nt + 1) * 512], po[:])
            nc.sync.dma_start(
                out_d[m][q0:q0 + 512, :].rearrange("(t p) f -> p t f", p=P),
                osb[:].rearrange("p (t f) -> p t f", f=DIM),
            )

        for i in range(len(steps) + LAG):
            if i < len(steps):
                m, other, qb, bi, kv, kt = steps[i]
                q0 = qb * 512
                dt_ps = ps_big.tile([P, 1024], F32, tag="dt")
                for h in range(2):
                    hs = slice(h * 64, (h + 1) * 64)
                    nc.tensor.matmul(
                        dt_ps[:, h * 512:(h + 1) * 512],
                        mm(kT[kv][hs, kt * P:(kt + 1) * P]),
                        mm(qT[m][hs, q0:q0 + 512]),
                        start=True, stop=True,
                        tile_position=(h * 64, 0),
                    )
                ex = exp_pool.tile([P, 1024], cdt, tag="ex")
                nc.scalar.activation(ex[:], dt_ps[:], EXP, scale=SCALE)
                exd[i] = ex
                # HAM-warming: standalone LDWEIGHTS stream SBUF->PE-array
                # activity without touching PSUM or serializing the dots
                # chain; they keep the clock-gate at 8/8 while the PE waits
                # on ACT. (bf16 only; f32 ldweights is a known walrus bug.)
                for _f in range(FILLERS):
                    nc.tensor.ldweights(kT[kv][0:64, kt * P:(kt + 1) * P])
            j = i - LAG
            if j >= 0:
                m, other, qb, bi, kv, kt = steps[j]
                if kt == 0:
                    avd[(m, qb, bi)] = {
                        h: ps_av.tile([65, 512], F32, name=f"av{bi}{h}", tag="av")
                        for h in range(2)}
                ex = exd.pop(j)
                for h in range(2):
                    nc.tensor.matmul(
                        avd[(m, qb, bi)][h][:],
                        mm(vsb[kv][:, kt * VW + h * 65: kt * VW + (h + 1) * 65]),
                        mm(ex[:, h * 512:(h + 1) * 512]),
                        start=(kt == 0), stop=(kt == KT - 1),
                    )
                if kt == KT - 1:
                    emit_branch_prep(m, qb, bi)
                    del avd[(m, qb, bi)]
                    if bi == 1:
                        emit_finalize(m, qb)

    nc.compile()
    return nc


_CACHE = {}


def get_nc(dtype_mode=DTYPE_MODE):
    if dtype_mode not in _CACHE:
        _CACHE[dtype_mode] = _build(dtype_mode)
    return _CACHE[dtype_mode]


def host_prep(inputs, dtype_mode=DTYPE_MODE):
    """Build per-core in_maps from the full f32 inputs."""
    npdt = NPBF16 if dtype_mode == "bf16" else np.float32
    img = np.asarray(inputs["img"], np.float32)
    clinic = np.asarray(inputs["clinic"], np.float32)
    W = {k: np.asarray(inputs[k], np.float32) for k in
         ("W_iq", "W_ik", "W_iv", "W_cq", "W_ck", "W_cv", "W_io", "W_co")}
    in_maps = []
    for core in range(8):
        b, hp = core // 4, core % 4
        sl = slice(hp * P, (hp + 1) * P)
        m = {
            "xt_a": np.ascontiguousarray(img[b].T).astype(npdt),
            "xt_b": np.ascontiguousarray(clinic[b].T).astype(npdt),
            "wq_a": np.ascontiguousarray(W["W_iq"][:, sl]).astype(npdt),
            "wk_a": np.ascontiguousarray(W["W_ik"][:, sl]).astype(npdt),
            "wv_a": np.ascontiguousarray(0.5 * W["W_iv"][:, sl]).astype(npdt),
            "wo_a": np.ascontiguousarray(W["W_io"][sl, :]).astype(npdt),
            "wq_b": np.ascontiguousarray(W["W_cq"][:, sl]).astype(npdt),
            "wk_b": np.ascontiguousarray(W["W_ck"][:, sl]).astype(npdt),
            "wv_b": np.ascontiguousarray(0.5 * W["W_cv"][:, sl]).astype(npdt),
            "wo_b": np.ascontiguousarray(W["W_co"][sl, :]).astype(npdt),
        }
        in_maps.append(m)
    return in_maps


def host_gather(results, inputs):
    """Sum the per-core partial outputs and add biases."""
    b_io = np.asarray(inputs["b_io"], np.float32)
    b_co = np.asarray(inputs["b_co"], np.float32)
    img_out = np.zeros((2, N, DIM), np.float32)
    clin_out = np.zeros((2, N, DIM), np.float32)
    for core in range(8):
        b = core // 4
        img_out[b] += np.asarray(results[core]["out_a"], np.float32)
        clin_out[b] += np.asarray(results[core]["out_b"], np.float32)
    img_out += b_io[None, None, :]
    clin_out += b_co[None, None, :]
    return img_out, clin_out


def kernel(**inputs):
    from concourse.bass_utils import run_bass_kernel_spmd
    nc = get_nc()
    in_maps = host_prep(inputs)
    res = run_bass_kernel_spmd(nc, in_maps, list(range(8)))
    return host_gather(res.results, inputs)


# revision 18
# speedup vs baseline: 1.1620x; 1.1620x over previous
"""Trainium2 Bass kernel for nn_CrossAttention_79362405696071.

Reference computation (per batch b):
  iq/ik/iv = img @ W_i{q,k,v}; cq/ck/cv = clinic @ W_c{q,k,v}   (8 heads x 64)
  k/v get one zero-padded key slot appended.
  img_out  = ((attend(iq,ck,cv) + attend(iq,ik,iv)) * 0.5) @ W_io + b_io
  clin_out = ((attend(cq,ik,iv) + attend(cq,ck,cv)) * 0.5) @ W_co + b_co

Sharding: 8 cores = batch(2) x head-pairs(4). Core c handles batch c//4 and
heads {2*(c%4), 2*(c%4)+1} (a 128-wide slice of the inner dim). Each core
computes both modalities' attention for its heads and the partial output
projection (contraction over its 128 inner dims). The 4 partials per batch
are summed on the host (host-side all-reduce), and host adds the bias.

Per-core structure (identical SPMD program, different data):
  xT[mod]    [512(dim), 2048(n)]  - host-transposed activations
  qT/kT      [128(pair-inner), 2048] = W_slice.T @ xT            (PE)
  v_ext      [2048(keys), per kt: v_h0(64)|1|v_h1(64)|1]  (v pre-scaled 0.5,
             with a ones column appended per head for the softmax denominator)
  dotsT      [128(key-tile), 2x512(h,q)] PSUM = k_h.T @ q_h (row-packed pair)
  expT       = exp(0.125 * dotsT)        (ACT, free-dim 1024)
  av_ext[bi][h] [65, 512] PSUM += [v_h|1].T @ expT_h  over 16 key tiles;
             row 64 accumulates the softmax denominator.
  combined_h = av_self_h * 1/(den_s+1) + av_cross_h * 1/(den_c+1)   (DVE)
             (+1 = zero-padded key slot: exp(0) in the denominator)
  out_part  += combined_h(128-col slices).T @ Wo_h  -> DRAM f32 partial
"""

import os
import sys

for _p in ("/opt/trn_rl_repo",):
    if _p not in sys.path:
        sys.path.insert(0, _p)

import numpy as np
import ml_dtypes
from contextlib import ExitStack

import concourse.bass as bass
import concourse.bacc as bacc
import concourse.tile as tile
import concourse.mybir as mybir

BF16 = mybir.dt.bfloat16
F32 = mybir.dt.float32
F32R = mybir.dt.float32r
EXP = mybir.ActivationFunctionType.Exp
NPBF16 = ml_dtypes.bfloat16

P = 128          # partitions
N = 2048         # sequence length (both modalities)
DIM = 512        # model dim
DC = DIM // P    # 4 dim chunks
KT = N // P      # 16 key tiles
QB = N // 512    # 4 query blocks of 512
VW = 130         # per-kt width in v_ext: 64 + 1 + 64 + 1
SCALE = 64 ** -0.5  # 1/8

# compute dtype mode: "bf16" or "f32r"
DTYPE_MODE = os.environ.get("KERNEL_DTYPE", "bf16")
FILLERS = int(os.environ.get("KERNEL_FILLERS", "0"))


def _build(dtype_mode):
    """Build + compile the single-core SPMD program. Returns nc."""
    cdt = BF16 if dtype_mode == "bf16" else F32
    mm = (lambda ap: ap) if dtype_mode == "bf16" else (lambda ap: ap.bitcast(F32R))

    nc = bacc.Bacc("TRN2", target_bir_lowering=False, debug=False)

    mods = ("a", "b")
    xt_d = {m: nc.declare_dram_parameter(f"xt_{m}", [DIM, N], cdt, isOutput=False).ap() for m in mods}
    wq_d = {m: nc.declare_dram_parameter(f"wq_{m}", [DIM, P], cdt, isOutput=False).ap() for m in mods}
    wk_d = {m: nc.declare_dram_parameter(f"wk_{m}", [DIM, P], cdt, isOutput=False).ap() for m in mods}
    wv_d = {m: nc.declare_dram_parameter(f"wv_{m}", [DIM, P], cdt, isOutput=False).ap() for m in mods}
    wo_d = {m: nc.declare_dram_parameter(f"wo_{m}", [P, DIM], cdt, isOutput=False).ap() for m in mods}
    out_d = {m: nc.declare_dram_parameter(f"out_{m}", [N, DIM], F32, isOutput=True).ap() for m in mods}

    with tile.TileContext(nc) as tc, ExitStack() as ctx:
        persist = ctx.enter_context(tc.tile_pool(name="persist", bufs=1))
        exp_pool = ctx.enter_context(tc.tile_pool(name="exp", bufs=12))
        misc = ctx.enter_context(tc.tile_pool(name="misc", bufs=2))
        ps_big = ctx.enter_context(tc.tile_pool(name="ps_big", bufs=3, space="PSUM"))
        ps_av = ctx.enter_context(tc.tile_pool(name="ps_av", bufs=2, space="PSUM"))

        # ---- load inputs to SBUF ----
        xt = {}   # [mod][chunk] -> [128, 2048]
        wq, wk, wv, wo = {}, {}, {}, {}
        for m in mods:
            xt[m] = []
            xr = xt_d[m].rearrange("(c p) n -> c p n", p=P)
            for c in range(DC):
                t = persist.tile([P, N], cdt, name=f"xt_{m}_{c}", tag=f"xt_{m}_{c}")
                nc.sync.dma_start(t[:], xr[c])
                xt[m].append(t)
            for wname, w, d in (("wq", wq, wq_d), ("wk", wk, wk_d), ("wv", wv, wv_d)):
                w[m] = persist.tile([P, DC * P], cdt, name=f"{wname}_{m}", tag=f"{wname}_{m}")
                nc.sync.dma_start(w[m][:].rearrange("p (c j) -> p c j", j=P),
                                  d[m].rearrange("(c p) j -> p c j", p=P))
            # wo stored as [64, 2, 512]: row j of head-half hh = wo[hh*64+j, :]
            wo[m] = persist.tile([64, 2, DIM], cdt, name=f"wo_{m}", tag=f"wo_{m}")
            nc.sync.dma_start(wo[m][:], wo_d[m].rearrange("(hh p) f -> p hh f", p=64))

        ones64 = persist.tile([1, 64], F32, name="ones64", tag="ones64")
        nc.vector.memset(ones64[:], 1.0)

        # ---- projections ----
        qT, kT, vsb = {}, {}, {}
        for m in mods:
            qT[m] = persist.tile([P, N], cdt, name=f"qT_{m}", tag=f"qT_{m}")
            kT[m] = persist.tile([P, N], cdt, name=f"kT_{m}", tag=f"kT_{m}")
            for w, dst in ((wq, qT), (wk, kT)):
                for nb in range(QB):
                    ps = ps_big.tile([P, 512], F32, tag="dt")
                    for c in range(DC):
                        nc.tensor.matmul(
                            ps[:],
                            mm(w[m][:, c * P:(c + 1) * P]),
                            mm(xt[m][c][:, nb * 512:(nb + 1) * 512]),
                            start=(c == 0), stop=(c == DC - 1),
                        )
                    nc.vector.tensor_copy(dst[m][:, nb * 512:(nb + 1) * 512], ps[:])
            # v_ext: [keys, per kt: v_h0|1|v_h1|1], v pre-scaled by 0.5 on host
            vsb[m] = persist.tile([P, KT * VW], cdt, name=f"v_{m}", tag=f"v_{m}")
            nc.vector.memset(vsb[m][:], 1.0)
            for kt in range(KT):
                ps = ps_big.tile([P, P], F32, tag="dt")
                for c in range(DC):
                    nc.tensor.matmul(
                        ps[:],
                        mm(xt[m][c][:, kt * P:(kt + 1) * P]),
                        mm(wv[m][:, c * P:(c + 1) * P]),
                        start=(c == 0), stop=(c == DC - 1),
                    )
                for h in range(2):
                    nc.vector.tensor_copy(
                        vsb[m][:, kt * VW + h * 65: kt * VW + h * 65 + 64],
                        ps[:, h * 64:(h + 1) * 64])

        # ---- attention + output projection: flat software pipeline ----
        # One stream over (mod, qb, branch, kt). The av/denominator matmuls
        # trail the dots+exp stream by LAG steps so the ACT engine (the
        # bottleneck) never stalls at branch/combo boundaries; normalize and
        # the output projection of combo j flow during combo j+1.
        LAG = 3
        steps = []
        for m in mods:
            other = "b" if m == "a" else "a"
            for qb in range(QB):
                for bi, kv in enumerate((m, other)):   # 0=self, 1=cross
                    for kt in range(KT):
                        steps.append((m, other, qb, bi, kv, kt))
        exd = {}     # step index -> ex tile
        avd = {}     # (m, qb, bi) -> {h: av psum tile}
        avsbd = {}   # (m, qb, bi) -> {h: drained sbuf tile}
        rdd = {}     # (m, qb, bi) -> {h: rd psum tile}

        def emit_branch_prep(m, qb, bi):
            # drain av psum -> SBUF, build reciprocal row, spread across
            # partitions for a lane-parallel reciprocal, gather back, and
            # K=1 ones-matmul broadcast to [64, 512]
            # (gpsimd.partition_broadcast is broken on HW; DVE cannot cross
            # partitions; +1 accounts for the zero-padded key slot)
            av = avd[(m, qb, bi)]
            avsbd[(m, qb, bi)] = {}
            dnrow = misc.tile([65, 1024], F32, name=f"dnrow{bi}", tag=f"dnrow{bi}")
            for h in range(2):
                nc.vector.tensor_scalar_add(
                    dnrow[64:65, h * 512:(h + 1) * 512], av[h][64:65, :], 1.0)
                t = misc.tile([64, 512], F32, name=f"avsb{bi}{h}", tag=f"avsb{bi}{h}")
                nc.vector.tensor_copy(t[:], av[h][0:64, :])
                avsbd[(m, qb, bi)][h] = t
            dsp = misc.tile([P, 8], F32, name=f"dsp{bi}", tag=f"dsp{bi}")
            nc.gpsimd.dma_start(dsp[:], dnrow[64:65, :])
            rsp = misc.tile([P, 8], F32, name=f"rsp{bi}", tag=f"rsp{bi}")
            nc.vector.reciprocal(rsp[:], dsp[:])
            r0 = misc.tile([1, 1024], F32, name=f"r0{bi}", tag=f"r0{bi}")
            nc.gpsimd.dma_start(r0[:], rsp[:])
            rdd[(m, qb, bi)] = {}
            for h in range(2):
                rd_ps = ps_big.tile([64, 512], F32, tag="dt")
                nc.tensor.matmul(rd_ps[:], ones64[:].bitcast(F32R),
                                 r0[:, h * 512:(h + 1) * 512].bitcast(F32R),
                                 start=True, stop=True)
                rdd[(m, qb, bi)][h] = rd_ps

        def emit_finalize(m, qb):
            avn = misc.tile([64, 2, 512], cdt, name="avn", tag="avn")
            for h in range(2):
                tmp0 = misc.tile([64, 512], F32, name="tmp0", tag="tmp0")
                tmp1 = misc.tile([64, 512], F32, name="tmp1", tag="tmp1")
                nc.vector.tensor_mul(tmp0[:], avsbd[(m, qb, 0)][h][:], rdd[(m, qb, 0)][h][:])
                nc.vector.tensor_mul(tmp1[:], avsbd[(m, qb, 1)][h][:], rdd[(m, qb, 1)][h][:])
                nc.vector.tensor_add(avn[:, h, :], tmp0[:], tmp1[:])
            del avsbd[(m, qb, 0)], avsbd[(m, qb, 1)], rdd[(m, qb, 0)], rdd[(m, qb, 1)]
            osb = misc.tile([P, 4 * 512], F32, name="osb", tag="osb")
            q0 = qb * 512
            for nt in range(4):
                po = ps_big.tile([P, 512], F32, tag="dt")
                for h in range(2):
                    nc.tensor.matmul(
                        po[:],
                        mm(avn[:, h, nt * P:(nt + 1) * P]),
                        mm(wo[m][:, h, :]),
                        start=(h == 0), stop=(h == 1),
                    )
                nc.vector.tensor_copy(osb[:, nt * 512:(nt + 1) * 512], po[:])
            nc.sync.dma_start(
                out_d[m][q0:q0 + 512, :].rearrange("(t p) f -> p t f", p=P),
                osb[:].rearrange("p (t f) -> p t f", f=DIM),
            )

        for i in range(len(steps) + LAG):
            if i < len(steps):
                m, other, qb, bi, kv, kt = steps[i]
                q0 = qb * 512
                dt_ps = ps_big.tile([P, 1024], F32, tag="dt")
                for h in range(2):
                    hs = slice(h * 64, (h + 1) * 64)
                    nc.tensor.matmul(
                        dt_ps[:, h * 512:(h + 1) * 512],
                        mm(kT[kv][hs, kt * P:(kt + 1) * P]),
                        mm(qT[m][hs, q0:q0 + 512]),
                        start=True, stop=True,
                        tile_position=(h * 64, 0),
                    )
                ex = exp_pool.tile([P, 1024], cdt, tag="ex")
                nc.scalar.activation(ex[:], dt_ps[:], EXP, scale=SCALE)
                exd[i] = ex
                # HAM-warming: standalone LDWEIGHTS stream SBUF->PE-array
                # activity without touching PSUM or serializing the dots
                # chain; they keep the clock-gate at 8/8 while the PE waits
                # on ACT. (bf16 only; f32 ldweights is a known walrus bug.)
                for _f in range(FILLERS):
                    nc.tensor.ldweights(kT[kv][0:64, kt * P:(kt + 1) * P])
            j = i - LAG
            if j >= 0:
                m, other, qb, bi, kv, kt = steps[j]
                if kt == 0:
                    avd[(m, qb, bi)] = {
                        h: ps_av.tile([65, 512], F32, name=f"av{bi}{h}", tag="av")
                        for h in range(2)}
                ex = exd.pop(j)
                for h in range(2):
                    nc.tensor.matmul(
                        avd[(m, qb, bi)][h][:],
                        mm(vsb[kv][:, kt * VW + h * 65: kt * VW + (h + 1) * 65]),
                        mm(ex[:, h * 512:(h + 1) * 512]),
                        start=(kt == 0), stop=(kt == KT - 1),
                    )
                if kt == KT - 1:
                    emit_branch_prep(m, qb, bi)
                    del avd[(m, qb, bi)]
                    if bi == 1:
                        emit_finalize(m, qb)

    nc.compile()
    return nc


_CACHE = {}


def get_nc(dtype_mode=DTYPE_MODE):
    if dtype_mode not in _CACHE:
        _CACHE[dtype_mode] = _build(dtype_mode)
    return _CACHE[dtype_mode]


def host_prep(inputs, dtype_mode=DTYPE_MODE):
    """Build per-core in_maps from the full f32 inputs."""
    npdt = NPBF16 if dtype_mode == "bf16" else np.float32
    img = np.asarray(inputs["img"], np.float32)
    clinic = np.asarray(inputs["clinic"], np.float32)
    W = {k: np.asarray(inputs[k], np.float32) for k in
         ("W_iq", "W_ik", "W_iv", "W_cq", "W_ck", "W_cv", "W_io", "W_co")}
    in_maps = []
    for core in range(8):
        b, hp = core // 4, core % 4
        sl = slice(hp * P, (hp + 1) * P)
        m = {
            "xt_a": np.ascontiguousarray(img[b].T).astype(npdt),
            "xt_b": np.ascontiguousarray(clinic[b].T).astype(npdt),
            "wq_a": np.ascontiguousarray(W["W_iq"][:, sl]).astype(npdt),
            "wk_a": np.ascontiguousarray(W["W_ik"][:, sl]).astype(npdt),
            "wv_a": np.ascontiguousarray(0.5 * W["W_iv"][:, sl]).astype(npdt),
            "wo_a": np.ascontiguousarray(W["W_io"][sl, :]).astype(npdt),
            "wq_b": np.ascontiguousarray(W["W_cq"][:, sl]).astype(npdt),
            "wk_b": np.ascontiguousarray(W["W_ck"][:, sl]).astype(npdt),
            "wv_b": np.ascontiguousarray(0.5 * W["W_cv"][:, sl]).astype(npdt),
            "wo_b": np.ascontiguousarray(W["W_co"][sl, :]).astype(npdt),
        }
        in_maps.append(m)
    return in_maps


def host_gather(results, inputs):
    """Sum the per-core partial outputs and add biases."""
    b_io = np.asarray(inputs["b_io"], np.float32)
    b_co = np.asarray(inputs["b_co"], np.float32)
    img_out = np.zeros((2, N, DIM), np.float32)
    clin_out = np.zeros((2, N, DIM), np.float32)
    for core in range(8):
        b = core // 4
        img_out[b] += np.asarray(results[core]["out_a"], np.float32)
        clin_out[b] += np.asarray(results[core]["out_b"], np.float32)
    img_out += b_io[None, None, :]
    clin_out += b_co[None, None, :]
    return img_out, clin_out


def kernel(**inputs):
    from concourse.bass_utils import run_bass_kernel_spmd
    nc = get_nc()
    in_maps = host_prep(inputs)
    res = run_bass_kernel_spmd(nc, in_maps, list(range(8)))
    return host_gather(res.results, inputs)


# revision 19
# speedup vs baseline: 1.5709x; 1.3518x over previous
"""Trainium2 Bass kernel for nn_CrossAttention_79362405696071.

Reference computation (per batch b):
  iq/ik/iv = img @ W_i{q,k,v}; cq/ck/cv = clinic @ W_c{q,k,v}   (8 heads x 64)
  k/v get one zero-padded key slot appended.
  img_out  = ((attend(iq,ck,cv) + attend(iq,ik,iv)) * 0.5) @ W_io + b_io
  clin_out = ((attend(cq,ik,iv) + attend(cq,ck,cv)) * 0.5) @ W_co + b_co

Sharding: 8 cores = batch(2) x head-pairs(4). Core c handles batch c//4 and
heads {2*(c%4), 2*(c%4)+1} (a 128-wide slice of the inner dim). Each core
computes both modalities' attention for its heads and the partial output
projection (contraction over its 128 inner dims). The 4 partials per batch
are summed on the host (host-side all-reduce), and host adds the bias.

Per-core structure (identical SPMD program, different data):
  xT[mod]    [512(dim), 2048(n)]  - host-transposed activations
  qT/kT      [128(pair-inner), 2048] = W_slice.T @ xT            (PE)
  v_ext      [2048(keys), per kt: v_h0(64)|1|v_h1(64)|1]  (v pre-scaled 0.5,
             with a ones column appended per head for the softmax denominator)
  dotsT      [128(key-tile), 2x512(h,q)] PSUM = k_h.T @ q_h (row-packed pair)
  expT       = exp(0.125 * dotsT)        (ACT, free-dim 1024)
  av_ext[bi][h] [65, 512] PSUM += [v_h|1].T @ expT_h  over 16 key tiles;
             row 64 accumulates the softmax denominator.
  combined_h = av_self_h * 1/(den_s+1) + av_cross_h * 1/(den_c+1)   (DVE)
             (+1 = zero-padded key slot: exp(0) in the denominator)
  out_part  += combined_h(128-col slices).T @ Wo_h  -> DRAM f32 partial
"""

import os
import sys

for _p in ("/opt/trn_rl_repo",):
    if _p not in sys.path:
        sys.path.insert(0, _p)

import numpy as np
import ml_dtypes
from contextlib import ExitStack

import concourse.bass as bass
import concourse.bacc as bacc
import concourse.tile as tile
import concourse.mybir as mybir

BF16 = mybir.dt.bfloat16
F32 = mybir.dt.float32
F32R = mybir.dt.float32r
EXP = mybir.ActivationFunctionType.Exp
NPBF16 = ml_dtypes.bfloat16

P = 128          # partitions
N = 2048         # sequence length (both modalities)
DIM = 512        # model dim
DC = DIM // P    # 4 dim chunks
KT = N // P      # 16 key tiles
QB = N // 512    # 4 query blocks of 512
VW = 130         # per-kt width in v_ext: 64 + 1 + 64 + 1
SCALE = 64 ** -0.5  # 1/8

# compute dtype mode: "bf16" or "f32r"
DTYPE_MODE = os.environ.get("KERNEL_DTYPE", "bf16")
FILLERS = int(os.environ.get("KERNEL_FILLERS", "0"))


def _build(dtype_mode):
    """Build + compile the single-core SPMD program. Returns nc."""
    cdt = BF16 if dtype_mode == "bf16" else F32
    mm = (lambda ap: ap) if dtype_mode == "bf16" else (lambda ap: ap.bitcast(F32R))

    nc = bacc.Bacc("TRN2", target_bir_lowering=False, debug=False)

    mods = ("a", "b")
    xt_d = {m: nc.declare_dram_parameter(f"xt_{m}", [DIM, N], cdt, isOutput=False).ap() for m in mods}
    wq_d = {m: nc.declare_dram_parameter(f"wq_{m}", [DIM, P], cdt, isOutput=False).ap() for m in mods}
    wk_d = {m: nc.declare_dram_parameter(f"wk_{m}", [DIM, P], cdt, isOutput=False).ap() for m in mods}
    wv_d = {m: nc.declare_dram_parameter(f"wv_{m}", [DIM, P], cdt, isOutput=False).ap() for m in mods}
    wo_d = {m: nc.declare_dram_parameter(f"wo_{m}", [P, DIM], cdt, isOutput=False).ap() for m in mods}
    out_d = {m: nc.declare_dram_parameter(f"out_{m}", [N, DIM], F32, isOutput=True).ap() for m in mods}

    with tile.TileContext(nc) as tc, ExitStack() as ctx:
        persist = ctx.enter_context(tc.tile_pool(name="persist", bufs=1))
        exp_pool = ctx.enter_context(tc.tile_pool(name="exp", bufs=12))
        misc = ctx.enter_context(tc.tile_pool(name="misc", bufs=2))
        ps_big = ctx.enter_context(tc.tile_pool(name="ps_big", bufs=2, space="PSUM"))
        ps_slow = ctx.enter_context(tc.tile_pool(name="ps_slow", bufs=2, space="PSUM"))
        ps_av = ctx.enter_context(tc.tile_pool(name="ps_av", bufs=2, space="PSUM"))

        # ---- load inputs to SBUF ----
        xt = {}   # [mod][chunk] -> [128, 2048]
        wq, wk, wv, wo = {}, {}, {}, {}
        for m in mods:
            xt[m] = []
            xr = xt_d[m].rearrange("(c p) n -> c p n", p=P)
            for c in range(DC):
                t = persist.tile([P, N], cdt, name=f"xt_{m}_{c}", tag=f"xt_{m}_{c}")
                nc.sync.dma_start(t[:], xr[c])
                xt[m].append(t)
            for wname, w, d in (("wq", wq, wq_d), ("wk", wk, wk_d), ("wv", wv, wv_d)):
                w[m] = persist.tile([P, DC * P], cdt, name=f"{wname}_{m}", tag=f"{wname}_{m}")
                nc.sync.dma_start(w[m][:].rearrange("p (c j) -> p c j", j=P),
                                  d[m].rearrange("(c p) j -> p c j", p=P))
            # wo stored as [64, 2, 512]: row j of head-half hh = wo[hh*64+j, :]
            wo[m] = persist.tile([64, 2, DIM], cdt, name=f"wo_{m}", tag=f"wo_{m}")
            nc.sync.dma_start(wo[m][:], wo_d[m].rearrange("(hh p) f -> p hh f", p=64))

        ones64 = persist.tile([1, 64], F32, name="ones64", tag="ones64")
        nc.vector.memset(ones64[:], 1.0)

        # ---- projections ----
        qT, kT, vsb = {}, {}, {}
        for m in mods:
            qT[m] = persist.tile([P, N], cdt, name=f"qT_{m}", tag=f"qT_{m}")
            kT[m] = persist.tile([P, N], cdt, name=f"kT_{m}", tag=f"kT_{m}")
            for w, dst in ((wq, qT), (wk, kT)):
                for nb in range(QB):
                    ps = ps_slow.tile([P, 512], F32, tag="slow")
                    for c in range(DC):
                        nc.tensor.matmul(
                            ps[:],
                            mm(w[m][:, c * P:(c + 1) * P]),
                            mm(xt[m][c][:, nb * 512:(nb + 1) * 512]),
                            start=(c == 0), stop=(c == DC - 1),
                        )
                    nc.vector.tensor_copy(dst[m][:, nb * 512:(nb + 1) * 512], ps[:])
            # v_ext: [keys, per kt: v_h0|1|v_h1|1], v pre-scaled by 0.5 on host
            vsb[m] = persist.tile([P, KT * VW], cdt, name=f"v_{m}", tag=f"v_{m}")
            nc.vector.memset(vsb[m][:], 1.0)
            for kt in range(KT):
                ps = ps_av.tile([P, P], F32, tag="av")
                for c in range(DC):
                    nc.tensor.matmul(
                        ps[:],
                        mm(xt[m][c][:, kt * P:(kt + 1) * P]),
                        mm(wv[m][:, c * P:(c + 1) * P]),
                        start=(c == 0), stop=(c == DC - 1),
                    )
                for h in range(2):
                    nc.vector.tensor_copy(
                        vsb[m][:, kt * VW + h * 65: kt * VW + h * 65 + 64],
                        ps[:, h * 64:(h + 1) * 64])

        # ---- attention + output projection: flat software pipeline ----
        # One stream over (mod, qb, branch, kt). The av/denominator matmuls
        # trail the dots+exp stream by LAG steps so the ACT engine (the
        # bottleneck) never stalls at branch/combo boundaries; normalize and
        # the output projection of combo j flow during combo j+1.
        LAG = 3
        steps = []
        for m in mods:
            other = "b" if m == "a" else "a"
            for qb in range(QB):
                for bi, kv in enumerate((m, other)):   # 0=self, 1=cross
                    for kt in range(KT):
                        steps.append((m, other, qb, bi, kv, kt))
        exd = {}     # step index -> ex tile
        avd = {}     # (m, qb, bi) -> {h: av psum tile}
        avsbd = {}   # (m, qb, bi) -> {h: drained sbuf tile}
        rdd = {}     # (m, qb, bi) -> {h: rd psum tile}

        def emit_branch_prep(m, qb, bi):
            # drain av psum -> SBUF, build reciprocal row, spread across
            # partitions for a lane-parallel reciprocal, gather back, and
            # K=1 ones-matmul broadcast to [64, 512]
            # (gpsimd.partition_broadcast is broken on HW; DVE cannot cross
            # partitions; +1 accounts for the zero-padded key slot)
            av = avd[(m, qb, bi)]
            avsbd[(m, qb, bi)] = {}
            dnrow = misc.tile([65, 1024], F32, name=f"dnrow{bi}", tag=f"dnrow{bi}")
            for h in range(2):
                nc.vector.tensor_scalar_add(
                    dnrow[64:65, h * 512:(h + 1) * 512], av[h][64:65, :], 1.0)
                t = misc.tile([64, 512], F32, name=f"avsb{bi}{h}", tag=f"avsb{bi}{h}")
                nc.vector.tensor_copy(t[:], av[h][0:64, :])
                avsbd[(m, qb, bi)][h] = t
            dsp = misc.tile([P, 8], F32, name=f"dsp{bi}", tag=f"dsp{bi}")
            nc.gpsimd.dma_start(dsp[:], dnrow[64:65, :])
            rsp = misc.tile([P, 8], F32, name=f"rsp{bi}", tag=f"rsp{bi}")
            nc.vector.reciprocal(rsp[:], dsp[:])
            r0 = misc.tile([1, 1024], F32, name=f"r0{bi}", tag=f"r0{bi}")
            nc.gpsimd.dma_start(r0[:], rsp[:])
            rdd[(m, qb, bi)] = {}
            for h in range(2):
                rd_ps = ps_slow.tile([64, 512], F32, tag="slow")
                nc.tensor.matmul(rd_ps[:], ones64[:].bitcast(F32R),
                                 r0[:, h * 512:(h + 1) * 512].bitcast(F32R),
                                 start=True, stop=True)
                rdd[(m, qb, bi)][h] = rd_ps

        def emit_finalize(m, qb):
            avn = misc.tile([64, 2, 512], cdt, name="avn", tag="avn")
            for h in range(2):
                tmp0 = misc.tile([64, 512], F32, name="tmp0", tag="tmp0")
                tmp1 = misc.tile([64, 512], F32, name="tmp1", tag="tmp1")
                nc.vector.tensor_mul(tmp0[:], avsbd[(m, qb, 0)][h][:], rdd[(m, qb, 0)][h][:])
                nc.vector.tensor_mul(tmp1[:], avsbd[(m, qb, 1)][h][:], rdd[(m, qb, 1)][h][:])
                nc.vector.tensor_add(avn[:, h, :], tmp0[:], tmp1[:])
            del avsbd[(m, qb, 0)], avsbd[(m, qb, 1)], rdd[(m, qb, 0)], rdd[(m, qb, 1)]
            osb = misc.tile([P, 4 * 512], F32, name="osb", tag="osb")
            q0 = qb * 512
            for nt in range(4):
                po = ps_slow.tile([P, 512], F32, tag="slow")
                for h in range(2):
                    nc.tensor.matmul(
                        po[:],
                        mm(avn[:, h, nt * P:(nt + 1) * P]),
                        mm(wo[m][:, h, :]),
                        start=(h == 0), stop=(h == 1),
                    )
                nc.vector.tensor_copy(osb[:, nt * 512:(nt + 1) * 512], po[:])
            nc.sync.dma_start(
                out_d[m][q0:q0 + 512, :].rearrange("(t p) f -> p t f", p=P),
                osb[:].rearrange("p (t f) -> p t f", f=DIM),
            )

        for i in range(len(steps) + LAG):
            if i < len(steps):
                m, other, qb, bi, kv, kt = steps[i]
                q0 = qb * 512
                dt_ps = ps_big.tile([P, 1024], F32, tag="dt")
                for h in range(2):
                    hs = slice(h * 64, (h + 1) * 64)
                    nc.tensor.matmul(
                        dt_ps[:, h * 512:(h + 1) * 512],
                        mm(kT[kv][hs, kt * P:(kt + 1) * P]),
                        mm(qT[m][hs, q0:q0 + 512]),
                        start=True, stop=True,
                        tile_position=(h * 64, 0),
                    )
                ex = exp_pool.tile([P, 1024], cdt, tag="ex")
                nc.scalar.activation(ex[:], dt_ps[:], EXP, scale=SCALE)
                exd[i] = ex
                # HAM-warming: standalone LDWEIGHTS stream SBUF->PE-array
                # activity without touching PSUM or serializing the dots
                # chain; they keep the clock-gate at 8/8 while the PE waits
                # on ACT. (bf16 only; f32 ldweights is a known walrus bug.)
                for _f in range(FILLERS):
                    nc.tensor.ldweights(kT[kv][0:64, kt * P:(kt + 1) * P])
            j = i - LAG
            if j >= 0:
                m, other, qb, bi, kv, kt = steps[j]
                if kt == 0:
                    avd[(m, qb, bi)] = {
                        h: ps_av.tile([65, 512], F32, name=f"av{bi}{h}", tag="av")
                        for h in range(2)}
                ex = exd.pop(j)
                for h in range(2):
                    nc.tensor.matmul(
                        avd[(m, qb, bi)][h][:],
                        mm(vsb[kv][:, kt * VW + h * 65: kt * VW + (h + 1) * 65]),
                        mm(ex[:, h * 512:(h + 1) * 512]),
                        start=(kt == 0), stop=(kt == KT - 1),
                    )
                if kt == KT - 1:
                    emit_branch_prep(m, qb, bi)
                    del avd[(m, qb, bi)]
                    if bi == 1:
                        emit_finalize(m, qb)

    nc.compile()
    return nc


_CACHE = {}


def get_nc(dtype_mode=DTYPE_MODE):
    if dtype_mode not in _CACHE:
        _CACHE[dtype_mode] = _build(dtype_mode)
    return _CACHE[dtype_mode]


def host_prep(inputs, dtype_mode=DTYPE_MODE):
    """Build per-core in_maps from the full f32 inputs."""
    npdt = NPBF16 if dtype_mode == "bf16" else np.float32
    img = np.asarray(inputs["img"], np.float32)
    clinic = np.asarray(inputs["clinic"], np.float32)
    W = {k: np.asarray(inputs[k], np.float32) for k in
         ("W_iq", "W_ik", "W_iv", "W_cq", "W_ck", "W_cv", "W_io", "W_co")}
    in_maps = []
    for core in range(8):
        b, hp = core // 4, core % 4
        sl = slice(hp * P, (hp + 1) * P)
        m = {
            "xt_a": np.ascontiguousarray(img[b].T).astype(npdt),
            "xt_b": np.ascontiguousarray(clinic[b].T).astype(npdt),
            "wq_a": np.ascontiguousarray(W["W_iq"][:, sl]).astype(npdt),
            "wk_a": np.ascontiguousarray(W["W_ik"][:, sl]).astype(npdt),
            "wv_a": np.ascontiguousarray(0.5 * W["W_iv"][:, sl]).astype(npdt),
            "wo_a": np.ascontiguousarray(W["W_io"][sl, :]).astype(npdt),
            "wq_b": np.ascontiguousarray(W["W_cq"][:, sl]).astype(npdt),
            "wk_b": np.ascontiguousarray(W["W_ck"][:, sl]).astype(npdt),
            "wv_b": np.ascontiguousarray(0.5 * W["W_cv"][:, sl]).astype(npdt),
            "wo_b": np.ascontiguousarray(W["W_co"][sl, :]).astype(npdt),
        }
        in_maps.append(m)
    return in_maps


def host_gather(results, inputs):
    """Sum the per-core partial outputs and add biases."""
    b_io = np.asarray(inputs["b_io"], np.float32)
    b_co = np.asarray(inputs["b_co"], np.float32)
    img_out = np.zeros((2, N, DIM), np.float32)
    clin_out = np.zeros((2, N, DIM), np.float32)
    for core in range(8):
        b = core // 4
        img_out[b] += np.asarray(results[core]["out_a"], np.float32)
        clin_out[b] += np.asarray(results[core]["out_b"], np.float32)
    img_out += b_io[None, None, :]
    clin_out += b_co[None, None, :]
    return img_out, clin_out


def kernel(**inputs):
    from concourse.bass_utils import run_bass_kernel_spmd
    nc = get_nc()
    in_maps = host_prep(inputs)
    res = run_bass_kernel_spmd(nc, in_maps, list(range(8)))
    return host_gather(res.results, inputs)
